# revision 9
# baseline (speedup 1.0000x reference)
"""Chamfer distance loss kernel for Trainium2 (8 NeuronCores, SPMD).

Problem: B=4 batches, N=M=8192 points, D=3.  loss = sum over batches of
  sum_i min_j ||c1_i - c2_j||^2  +  sum_j min_i ||c2_j - c1_i||^2

Sharding: the 4 batches x 2 directions give exactly 8 independent
(A-cloud, B-cloud) brute-force nearest-neighbor tasks - one per core.
No collectives needed.

Per core, the [8192 x 8192] squared-distance matrix is produced by the
TensorEngine as a single matmul over K=24 "augmented" features:
  d[i,j] = sq_i + sq_j - 2*(x_i x_j + y_i y_j + z_i z_j)
Each fp32 scalar is split exactly into 3 bf16 terms (24 = 8+8+8 mantissa
bits), and the products are expanded so every kept bf16*bf16 product is
exact in the fp32 PSUM accumulation; dropped cross terms are below one
fp32 ulp of the inputs.  This gives fp32-quality distances at the PE's
full bf16 rate (fp32 matmul would be 4x slower).

The VectorEngine then does a running min-reduce straight out of PSUM
([128 x 2048] per op, i.e. 4 matmul banks per reduce), and a final
min+sum produces a [128,1] per-core partial, summed on the host.
"""

import numpy as np

try:
    import concourse.bass as bass  # noqa: F401
except ImportError:  # harness may run with a bare sys.path
    import sys

    for p in ("/root/.axon_site/_ro/trn_rl_repo", "/opt/trn_rl_repo", "/opt/pypackages"):
        if p not in sys.path:
            sys.path.append(p)
    import concourse.bass as bass  # noqa: F401

import ml_dtypes

B, N, M, D = 4, 8192, 8192, 3
KFEAT = 24
NCORES = 8
PT = 128          # A points per row-tile (PSUM partitions)
BLK = 512         # B points per matmul (one fp32 PSUM bank)
GROUP_BLKS = 4    # matmul banks per vector reduce ([128, 2048])

_BF16 = ml_dtypes.bfloat16


def _split3(v):
    """Exact 3-way bf16 split of fp32: v == vh + vl + vll (8+8+8 mantissa)."""
    vh = v.astype(_BF16).astype(np.float32)
    r = v - vh
    vl = r.astype(_BF16).astype(np.float32)
    vll = (r - vl).astype(_BF16).astype(np.float32)
    return vh, vl, vll


def _features(A, Bc):
    """Build the K=24 augmented feature matrices.

    A: [n,3] row-side cloud, Bc: [m,3] column-side cloud.
    Returns FA [24,n] bf16, FB [24,m] bf16 with
      FA[:,i] . FB[:,j] ~= ||A_i - B_j||^2  (fp32-accurate)
    """
    A = np.asarray(A, np.float32)
    Bc = np.asarray(Bc, np.float32)
    sqA = (A * A).sum(-1, dtype=np.float32)
    sqB = (Bc * Bc).sum(-1, dtype=np.float32)
    FA, FB = [], []
    for k in range(3):
        ah, al, all_ = _split3(A[:, k])
        bh, bl, bll = _split3(Bc[:, k])
        # kept products: hh, hl, lh, ll, h*ll, ll*h  (each exact in fp32)
        FA += [ah, ah, al, al, ah, all_]
        FB += [-2 * bh, -2 * bl, -2 * bh, -2 * bl, -2 * bll, -2 * bh]
    a1, a2, a3 = _split3(sqA)
    ones_m = np.ones_like(sqB)
    FA += [a1, a2, a3]
    FB += [ones_m, ones_m, ones_m]
    b1, b2, b3 = _split3(sqB)
    ones_n = np.ones_like(sqA)
    FA += [ones_n, ones_n, ones_n]
    FB += [b1, b2, b3]
    fa = np.stack(FA, 0).astype(_BF16)
    fb = np.stack(FB, 0).astype(_BF16)
    return fa, fb


def _split_waits(nc, max_waits=1):
    """Walrus in this toolchain accepts at most one sync-wait command per
    instruction; Tile fuses several.  Hoist extra waits into standalone
    event-semaphore instructions right before the owner (same engine, so
    program order preserves semantics)."""
    from concourse import mybir

    for f in nc.m.functions:
        for bb in f.blocks:
            new_insts = []
            for ins in bb.instructions:
                si = ins.sync_info
                waits = list(si.on_wait) if si and si.on_wait else []
                if len(waits) > max_waits:
                    extra, keep = waits[:-max_waits], waits[-max_waits:]
                    for k, w in enumerate(extra):
                        ev = mybir.InstEventSemaphore(
                            name=f"{ins.name}-evw{k}", ins=[], outs=[]
                        )
                        ev.engine = ins.engine
                        ev.sync_info = mybir.SyncInfo(on_wait=[w], on_update=[])
                        new_insts.append(ev)
                    ins.sync_info = mybir.SyncInfo(
                        on_wait=keep, on_update=list(si.on_update)
                    )
                new_insts.append(ins)
            bb.instructions[:] = new_insts
    return nc


def build_nc(n_a=N, n_b=M, reps=1, group_blks=GROUP_BLKS, psum_bufs=2):
    """Build the per-core Bass program (SPMD: same program, per-core data)."""
    import concourse.tile as tile
    from concourse import mybir

    row_tiles = n_a // PT
    nblk = n_b // BLK
    ngroups = nblk // group_blks
    assert n_a % PT == 0 and n_b % (BLK * group_blks) == 0

    nc = bass.Bass("TRN2", target_bir_lowering=False, debug=False, num_devices=NCORES)
    # one packed input tensor -> a single input DMA (keeps the kernel-tail
    # drain within walrus's sync-wait-command limit)
    feat_d = nc.dram_tensor(
        "feat", [KFEAT, n_a + n_b], mybir.dt.bfloat16, kind="ExternalInput"
    )
    out_d = nc.dram_tensor("out", [PT, 1], mybir.dt.float32, kind="ExternalOutput")

    with tile.TileContext(nc) as tc:
        with (
            tc.tile_pool(name="const", bufs=1) as cpool,
            tc.tile_pool(name="psum", bufs=psum_bufs, space="PSUM") as ppool,
            tc.tile_pool(name="accum", bufs=1) as apool,
        ):
            feat = cpool.tile([KFEAT, n_a + n_b], mybir.dt.bfloat16)
            nc.sync.dma_start(feat[:], feat_d[:])
            af = feat[:, :n_a]
            bf = feat[:, n_a:]

            mins = apool.tile([PT, row_tiles * ngroups], mybir.dt.float32)
            m2 = apool.tile([PT, row_tiles], mybir.dt.float32)
            res = apool.tile([PT, 1], mybir.dt.float32)

            for _ in range(reps):
                for t in range(row_tiles):
                    lhsT = af[:, t * PT:(t + 1) * PT]
                    for g in range(ngroups):
                        ps = ppool.tile([PT, BLK * group_blks], mybir.dt.float32)
                        for q in range(group_blks):
                            j = g * group_blks + q
                            nc.tensor.matmul(
                                ps[:, q * BLK:(q + 1) * BLK],
                                lhsT,
                                bf[:, j * BLK:(j + 1) * BLK],
                                start=True,
                                stop=True,
                            )
                        nc.vector.tensor_reduce(
                            mins[:, t * ngroups + g: t * ngroups + g + 1],
                            ps[:],
                            axis=mybir.AxisListType.X,
                            op=mybir.AluOpType.min,
                        )
                nc.vector.tensor_reduce(
                    m2[:],
                    mins[:].rearrange("p (t g) -> p t g", g=ngroups),
                    axis=mybir.AxisListType.X,
                    op=mybir.AluOpType.min,
                )
                nc.vector.tensor_reduce(
                    res[:],
                    m2[:],
                    axis=mybir.AxisListType.X,
                    op=mybir.AluOpType.add,
                )
            nc.sync.dma_start(out_d[:], res[:])
    return _split_waits(nc)


def make_in_maps(cloud1, cloud2):
    """Per-core inputs: core 2b+0 handles (c1[b]->c2[b]), 2b+1 the reverse."""
    in_maps = []
    for b in range(B):
        for A, Bc in ((cloud1[b], cloud2[b]), (cloud2[b], cloud1[b])):
            fa, fb = _features(A, Bc)
            in_maps.append({"feat": np.concatenate([fa, fb], axis=1)})
    return in_maps


_NC_CACHE = {}


def kernel(cloud1, cloud2):
    from concourse.bass_utils import run_bass_kernel_spmd

    cloud1 = np.asarray(cloud1, np.float32)
    cloud2 = np.asarray(cloud2, np.float32)
    assert cloud1.shape == (B, N, D) and cloud2.shape == (B, M, D)

    if "nc" not in _NC_CACHE:
        _NC_CACHE["nc"] = build_nc()
    nc = _NC_CACHE["nc"]

    in_maps = make_in_maps(cloud1, cloud2)
    results = run_bass_kernel_spmd(nc, in_maps, list(range(NCORES))).results
    total = 0.0
    for c in range(NCORES):
        total += float(results[c]["out"].astype(np.float64).sum())
    return np.array(total, dtype=np.float32)
